# revision 17
# baseline (speedup 1.0000x reference)
"""Trainium2 Bass kernel for nn_CrossAttention (B=8, N=4096, S=512, D=512, H=8).

Sharding: data-parallel over batch — each of the 8 NeuronCores computes the
full cross-attention for one batch element. No collectives needed.

v2 (all matmuls bf16, mask-free softmax):
  - host zeroes masked context columns, so masked keys give exp(0)=1 and
    masked values contribute nothing to PV; the softmax denominator is
    corrected by subtracting the per-batch masked count M_b. This removes
    the mask bias from the exp, letting one ACT instruction cover a
    [128, 1024] two-bank PSUM region (two heads per scores group).
  - scores per (pair c, sc): two K=64 matmuls in PE row groups 0-63/64-127
    (concurrent), writing halves of a [128, 1024] PSUM tile; one Exp.
  - denominators via M=1 ones-matmuls, 4 heads col-tiled at array columns
    {0,32,64,96} (concurrent), accumulated over sc.
  - PV col-tiled: even head -> PSUM rows 0:64 (cols 0-63), odd head ->
    rows 64:128 (cols 64-127), concurrent, accumulated over sc.
  - den rows gathered with one strided-partition DVE copy (+ -M_b add),
    reciprocal_approx_fast, cast to bf16, broadcast across partitions with
    a K=4 sel-matmul; normalize = one dual-PSUM DVE multiply per pair.
  - q-proj for tile t+1 is emitted before out-proj of tile t so the PE
    FIFO always has independent work while the softmax chain completes.
  - PSUM budget: ps_s [128,1024] x2 (4 banks) + pair x2 + qy x2 = 8 banks.
"""

import os

import numpy as np

try:
    import concourse.bass as bass
except ImportError:
    import sys

    sys.path.insert(0, "/opt/trn_rl_repo")
    import concourse.bass as bass

from contextlib import ExitStack

import concourse.mybir as mybir
import concourse.tile as tile
from concourse.bass import ts

B, N, S, D, H = 8, 4096, 512, 512, 8
HD = D // H  # 64
SCALE = HD**-0.5
P = 128
IC = D // P  # 4 chunks of the contraction/feature dims
SC = S // P  # 4 chunks of the context length
NT = 512  # queries per outer tile
NTILES = N // NT  # 8
NSUB = NT // P  # 4

f32 = mybir.dt.float32

MMDT_NAME = os.environ.get("KMMDT", "bfloat16")


def _np_mm(mmdt):
    return np.dtype(mybir.dt.np(mmdt))


def _split_multi_waits(nc: bass.Bass) -> None:
    """This walrus toolchain accepts at most ONE sync-wait per instruction
    ("Too many sync wait commands" in setupSyncWait, seen for MM/LW, NoOp,
    and DMA structs alike). Hoist all but the last wait of any instruction
    onto a chain of same-engine InstNoOps spliced immediately before it —
    same program position, so synchronization semantics are unchanged."""
    eng_map = {
        mybir.EngineType.PE: lambda: nc.tensor,
        mybir.EngineType.Activation: lambda: nc.scalar,
        mybir.EngineType.DVE: lambda: nc.vector,
        mybir.EngineType.Pool: lambda: nc.gpsimd,
        mybir.EngineType.SP: lambda: nc.sync,
    }
    for fn in nc.m.functions:
        blocks = fn.blocks
        for bb in blocks:
            insts = list(bb.instructions)
            out = []
            changed = False
            for inst in insts:
                si = inst.sync_info
                if (
                    si is not None
                    and len(si.on_wait) > 1
                    and inst.engine in eng_map
                ):
                    waits = list(si.on_wait)
                    for w in waits[:-1]:  # one nop per excess wait
                        nop = eng_map[inst.engine]().nop(nofuse=True).ins
                        # the nop was appended to whatever block is current;
                        # strip it from there before splicing it in place
                        for bb2 in blocks:
                            lst = list(bb2.instructions)
                            if any(x.name == nop.name for x in lst):
                                bb2.instructions = [
                                    x for x in lst if x.name != nop.name
                                ]
                                if bb2 is bb:
                                    insts = [
                                        x for x in insts if x.name != nop.name
                                    ]
                        nop.sync_info = mybir.SyncInfo(
                            on_wait=[w], on_update=[]
                        )
                        out.append(nop)
                    inst.sync_info = mybir.SyncInfo(
                        on_wait=waits[-1:], on_update=list(si.on_update)
                    )
                    changed = True
                out.append(inst)
            if changed:
                bb.instructions = out


def _build_nc(mmdt_name: str, has_bq, has_bk, has_bv, has_bp) -> bass.Bass:
    mmdt = getattr(mybir.dt, mmdt_name)
    nc = bass.Bass()

    xT = nc.dram_tensor("xT", [D, N], mmdt, kind="ExternalInput")
    ctxT = nc.dram_tensor("ctxT", [D, S], mmdt, kind="ExternalInput")
    wqT = nc.dram_tensor("wqT", [D, D], mmdt, kind="ExternalInput")
    wkT = nc.dram_tensor("wkT", [D, D], mmdt, kind="ExternalInput")
    wvT = nc.dram_tensor("wvT", [D, D], mmdt, kind="ExternalInput")
    wpT = nc.dram_tensor("wpT", [D, D], mmdt, kind="ExternalInput")
    bq = nc.dram_tensor("bq", [D, 1], f32, kind="ExternalInput")
    bk = nc.dram_tensor("bk", [D, 1], f32, kind="ExternalInput")
    bv = nc.dram_tensor("bv", [1, D], mmdt, kind="ExternalInput")
    bp = nc.dram_tensor("bp", [1, D], mmdt, kind="ExternalInput")
    mneg = nc.dram_tensor("mneg", [2, NT], mmdt, kind="ExternalInput")
    seld = nc.dram_tensor("seld", [2, P, P], mmdt, kind="ExternalInput")
    y = nc.dram_tensor("y", [N, D], mmdt, kind="ExternalOutput")

    ch = lambda dram: dram.rearrange("(c p) o -> p c o", p=P)  # [P, IC, D]

    with tile.TileContext(nc) as tc, ExitStack() as ctx:
        const = ctx.enter_context(tc.tile_pool(name="const", bufs=1))
        work = ctx.enter_context(tc.tile_pool(name="work", bufs=3))
        epool = ctx.enter_context(tc.tile_pool(name="epool", bufs=20))
        dpool = ctx.enter_context(tc.tile_pool(name="dpool", bufs=4))
        ypool = ctx.enter_context(tc.tile_pool(name="ypool", bufs=4))
        psum = ctx.enter_context(tc.tile_pool(name="psum", bufs=1, space="PSUM"))

        # ---- persistent tiles -------------------------------------------
        wq_t = const.tile([P, IC, D], mmdt)
        x0_t = const.tile([P, IC, NT], mmdt)  # tile-0 x, loaded first
        wk_t = const.tile([P, IC, D], mmdt)
        wv_t = const.tile([P, IC, D], mmdt)
        wp_t = const.tile([P, IC, D], mmdt)
        ctx_t = const.tile([P, IC, S], mmdt)
        selA_t = const.tile([P, P], mmdt)
        selB_t = const.tile([P, P], mmdt)
        mneg_t = const.tile([2, NT], mmdt)
        ones1_t = const.tile([P, 1], mmdt)
        ones2_t = const.tile([2, 1], mmdt)
        # wq + x0 first so q-proj can start while the rest stream in
        nc.sync.dma_start(wq_t[:], ch(wqT))
        nc.sync.dma_start(
            x0_t[:], xT[:, 0:NT].rearrange("(c p) n -> p c n", p=P)
        )
        nc.sync.dma_start(ctx_t[:], ch(ctxT))
        nc.sync.dma_start(wk_t[:], ch(wkT))
        nc.sync.dma_start(wv_t[:], ch(wvT))
        nc.sync.dma_start(wp_t[:], ch(wpT))
        nc.sync.dma_start(selA_t[:], seld[0])
        nc.sync.dma_start(selB_t[:], seld[1])
        nc.sync.dma_start(mneg_t[:], mneg[:])
        nc.vector.memset(ones1_t[:], 1.0)
        nc.vector.memset(ones2_t[:], 1.0)

        if has_bq:
            bq_t = const.tile([P, IC, 1], f32)
            nc.sync.dma_start(bq_t[:], bq.rearrange("(c p) o -> p c o", p=P))
        if has_bk:
            bk_t = const.tile([P, IC, 1], f32)
            nc.sync.dma_start(bk_t[:], bk.rearrange("(c p) o -> p c o", p=P))
        if has_bv:
            bv_t = const.tile([P, D], mmdt)
            nc.sync.dma_start(bv_t[:], bv.to_broadcast((P, D)))
        if has_bp:
            bp_t = const.tile([P, D], mmdt)
            nc.sync.dma_start(bp_t[:], bp.to_broadcast((P, D)))

        kT_t = const.tile([P, IC, S], mmdt)  # feature-major keys
        v_t = const.tile([P, SC, H, HD], mmdt)  # token-major values

        # ---- q-proj helper ----------------------------------------------
        def emit_qproj(x_tile, q_tile):
            for oc in range(IC):
                ps = psum.tile([P, NT], f32, tag="ps_qy", bufs=2)
                for i in range(IC):
                    nc.tensor.matmul(
                        ps[:],
                        wq_t[:, i, ts(oc, P)],
                        x_tile[:, i, :],
                        start=(i == 0),
                        stop=(i == IC - 1),
                    )
                if has_bq:
                    nc.vector.tensor_scalar_add(
                        q_tile[:, oc, :], ps[:], bq_t[:, oc, :]
                    )
                else:
                    nc.vector.tensor_copy(q_tile[:, oc, :], ps[:])

        # q-proj for tile 0 first: PE work that only needs wq + x0
        qT_cur = work.tile([P, IC, NT], mmdt, tag="qT")
        emit_qproj(x0_t, qT_cur)

        # ---- kv projections (once per core) -----------------------------
        for kc in range(IC):  # dk chunks -> kT
            ps = psum.tile([P, S], f32, tag="ps_qy", bufs=2)
            for i in range(IC):
                nc.tensor.matmul(
                    ps[:],
                    wk_t[:, i, ts(kc, P)],
                    ctx_t[:, i, :],
                    start=(i == 0),
                    stop=(i == IC - 1),
                )
            if has_bk:
                nc.vector.tensor_scalar_add(kT_t[:, kc, :], ps[:], bk_t[:, kc, :])
            else:
                nc.vector.tensor_copy(kT_t[:, kc, :], ps[:])

        for sc in range(SC):  # s chunks -> v (token-major)
            ps = psum.tile([P, D], f32, tag="ps_qy", bufs=2)
            for i in range(IC):
                nc.tensor.matmul(
                    ps[:],
                    ctx_t[:, i, ts(sc, P)],
                    wv_t[:, i, :],
                    start=(i == 0),
                    stop=(i == IC - 1),
                )
            if has_bv:
                nc.vector.tensor_add(v_t[:, sc, :, :], ps[:], bv_t[:])
            else:
                nc.vector.tensor_copy(v_t[:, sc, :, :], ps[:])

        # ---- main loop over query tiles ---------------------------------
        # Software-pipelined one tile deep: out-proj of tile t-1 is emitted
        # after the scores of tile t, so the PE FIFO never stalls on the
        # den/recip/broadcast/normalize tail of the current tile.
        prev = None  # (ot_t of previous tile, its tile index)
        for t in range(NTILES):
            es = {}
            rdens = {}
            ot_t = work.tile([P, IC, NT], mmdt, tag="ot")
            if t + 1 < NTILES:
                x_next = work.tile([P, IC, NT], mmdt, tag="xT")
                nc.sync.dma_start(
                    x_next[:],
                    xT[:, ts(t + 1, NT)].rearrange("(c p) n -> p c n", p=P),
                )

            # scores + exp for the whole tile (16 two-head groups)
            for c in range(IC):
                for sc in range(SC):
                    ps_s = psum.tile([P, 2 * NT], f32, tag="ps_s", bufs=2)
                    for par in (0, 1):  # PE row groups 0-63 / 64-127
                        pslc = slice(par * HD, (par + 1) * HD)
                        nc.tensor.matmul(
                            ps_s[:, par * NT : (par + 1) * NT],
                            kT_t[pslc, c, ts(sc, P)],
                            qT_cur[pslc, c, :],
                            start=True,
                            stop=True,
                        )
                    e = epool.tile([P, 2 * NT], mmdt, tag="e")
                    nc.scalar.activation(
                        e[:],
                        ps_s[:],
                        mybir.ActivationFunctionType.Exp,
                        scale=SCALE,
                    )
                    es[c, sc] = e

            # out-proj of the PREVIOUS tile: ready PE work while exps drain
            if prev is not None:
                pot, pt = prev
                for ns in range(NSUB):
                    ps_y = psum.tile([P, D], f32, tag="ps_qy", bufs=2)
                    for c in range(IC):
                        nc.tensor.matmul(
                            ps_y[:],
                            pot[:, c, ts(ns, P)],
                            wp_t[:, c, :],
                            start=(c == 0),
                            stop=(c == IC - 1),
                        )
                    y_t = ypool.tile([P, D], mmdt, tag="y")
                    if has_bp:
                        nc.vector.tensor_add(y_t[:], ps_y[:], bp_t[:])
                    else:
                        nc.vector.tensor_copy(y_t[:], ps_y[:])
                    nc.sync.dma_start(
                        y[pt * NT + ns * P : pt * NT + (ns + 1) * P, :], y_t[:]
                    )

            # q-proj for tile t+1: more ready PE work
            if t + 1 < NTILES:
                qT_next = work.tile([P, IC, NT], mmdt, tag="qT")
                emit_qproj(x_next, qT_next)

            # denominators + PV + normalize per head-quad
            for quad in range(2):
                ps_den = psum.tile([P, NT], f32, tag="ps_qy", bufs=2)
                nc.vector.memset(ps_den[:], 0.0)
                for sc in range(SC):
                    for j in range(4):
                        h = 4 * quad + j
                        nc.tensor.matmul(
                            ps_den[32 * j : 32 * j + 1, :],
                            ones1_t[:],
                            es[h // 2, sc][:, (h % 2) * NT : (h % 2 + 1) * NT],
                            start=(sc == 0),
                            stop=False,
                            tile_position=(0, 32 * j),
                        )
                for j in range(4):
                    # subtract masked count: -M_b = ones2.T @ (-M_b/2 rows)
                    nc.tensor.matmul(
                        ps_den[32 * j : 32 * j + 1, :],
                        ones2_t[:],
                        mneg_t[:],
                        start=False,
                        stop=True,
                        tile_position=(0, 32 * j),
                    )
                den_sb = dpool.tile([P, NT], f32, tag="den")
                nc.vector.tensor_scalar_max(den_sb[:], ps_den[:], 1e-20)
                rdenf = dpool.tile([P, NT], f32, tag="rdenf")
                nc.vector.reciprocal(rdenf[:], den_sb[:])
                rden = dpool.tile([P, NT], mmdt, tag="rden")
                nc.vector.tensor_copy(rden[:], rdenf[:])
                rdens[quad] = rden

                pairs = {}
                for c in (2 * quad, 2 * quad + 1):
                    ps_pair = psum.tile([P, NT], f32, tag="ps_pair", bufs=2)
                    for sc in range(SC):
                        for par in (0, 1):
                            nc.tensor.matmul(
                                ps_pair[par * HD : (par + 1) * HD, :],
                                v_t[:, sc, 2 * c + par, :],
                                es[c, sc][:, par * NT : (par + 1) * NT],
                                start=(sc == 0),
                                stop=(sc == SC - 1),
                            )
                    pairs[c] = ps_pair
                for c in (2 * quad, 2 * quad + 1):
                    ps_bc = psum.tile([P, NT], f32, tag="ps_qy", bufs=2)
                    nc.tensor.matmul(
                        ps_bc[:],
                        (selA_t if c % 2 == 0 else selB_t)[:],
                        rdens[quad][:],
                        start=True,
                        stop=True,
                    )
                    bc_sb = dpool.tile([P, NT], mmdt, tag="bc")
                    nc.vector.tensor_copy(bc_sb[:], ps_bc[:])
                    nc.vector.tensor_mul(ot_t[:, c, :], pairs[c][:], bc_sb[:])

            prev = (ot_t, t)
            if t + 1 < NTILES:
                qT_cur = qT_next

        # drain: out-proj of the last tile
        pot, pt = prev
        for ns in range(NSUB):
            ps_y = psum.tile([P, D], f32, tag="ps_qy", bufs=2)
            for c in range(IC):
                nc.tensor.matmul(
                    ps_y[:],
                    pot[:, c, ts(ns, P)],
                    wp_t[:, c, :],
                    start=(c == 0),
                    stop=(c == IC - 1),
                )
            y_t = ypool.tile([P, D], mmdt, tag="y")
            if has_bp:
                nc.vector.tensor_add(y_t[:], ps_y[:], bp_t[:])
            else:
                nc.vector.tensor_copy(y_t[:], ps_y[:])
            nc.sync.dma_start(
                y[pt * NT + ns * P : pt * NT + (ns + 1) * P, :], y_t[:]
            )

    _split_multi_waits(nc)
    return nc


_NC_CACHE: dict = {}


def _get_nc(flags):
    if flags not in _NC_CACHE:
        _NC_CACHE[flags] = _build_nc(*flags)
    return _NC_CACHE[flags]


def _prep_in_maps(x, context, context_mask, wq, bq, wkv, bkv, wp, bp, mmdt_name=None):
    if mmdt_name is None:
        mmdt_name = MMDT_NAME
    np_mm = _np_mm(getattr(mybir.dt, mmdt_name))
    cvt = lambda a: np.ascontiguousarray(a).astype(np_mm, copy=False)
    wqT = cvt(wq.T)
    wkT = cvt(wkv[:D].T)
    wvT = cvt(wkv[D:].T)
    wpT = cvt(wp.T)
    bq_c = np.ascontiguousarray(bq.reshape(D, 1), dtype=np.float32)
    bk_c = np.ascontiguousarray(bkv[:D].reshape(D, 1), dtype=np.float32)
    bv_r = cvt(bkv[D:].reshape(1, D))
    bp_r = cvt(bp.reshape(1, D))
    sel = np.zeros((2, P, P), np.float32)
    sel[0, 0, 0:HD] = 1.0   # even pair of a quad: den rows 0 / 32
    sel[0, 32, HD:P] = 1.0
    sel[1, 64, 0:HD] = 1.0  # odd pair of a quad: den rows 64 / 96
    sel[1, 96, HD:P] = 1.0
    sel = cvt(sel)
    flags = (
        mmdt_name,
        bool(np.any(bq != 0)),
        bool(np.any(bkv[:D] != 0)),
        bool(np.any(bkv[D:] != 0)),
        bool(np.any(bp != 0)),
    )
    in_maps = []
    for b in range(B):
        ctx0 = np.where(context_mask[b][:, None], 0.0, context[b]).astype(
            context.dtype
        )
        m_b = float(context_mask[b].sum())
        in_maps.append(
            {
                "xT": cvt(x[b].T),
                "ctxT": cvt(ctx0.T),
                "wqT": wqT,
                "wkT": wkT,
                "wvT": wvT,
                "wpT": wpT,
                "bq": bq_c,
                "bk": bk_c,
                "bv": bv_r,
                "bp": bp_r,
                "mneg": cvt(np.full((2, NT), -m_b / 2.0, np.float32)),
                "seld": sel,
            }
        )
    return in_maps, flags


def kernel(x, context, context_mask, wq, bq, wkv, bkv, wp, bp):
    from concourse.bass_utils import run_bass_kernel_spmd

    in_maps, flags = _prep_in_maps(
        x, context, context_mask, wq, bq, wkv, bkv, wp, bp
    )
    nc = _get_nc(flags)
    res = run_bass_kernel_spmd(nc, in_maps, list(range(B)))
    return np.stack(
        [np.asarray(res.results[b]["y"]).astype(np.float32) for b in range(B)],
        axis=0,
    )


# revision 18
# speedup vs baseline: 1.2158x; 1.2158x over previous
"""Trainium2 Bass kernel for nn_CrossAttention (B=8, N=4096, S=512, D=512, H=8).

Sharding: data-parallel over batch — each of the 8 NeuronCores computes the
full cross-attention for one batch element. No collectives needed.

v2 (all matmuls bf16, mask-free softmax):
  - host zeroes masked context columns, so masked keys give exp(0)=1 and
    masked values contribute nothing to PV; the softmax denominator is
    corrected by subtracting the per-batch masked count M_b. This removes
    the mask bias from the exp, letting one ACT instruction cover a
    [128, 1024] two-bank PSUM region (two heads per scores group).
  - scores per (pair c, sc): two K=64 matmuls in PE row groups 0-63/64-127
    (concurrent), writing halves of a [128, 1024] PSUM tile; one Exp.
  - denominators via M=1 ones-matmuls, 4 heads col-tiled at array columns
    {0,32,64,96} (concurrent), accumulated over sc.
  - PV col-tiled: even head -> PSUM rows 0:64 (cols 0-63), odd head ->
    rows 64:128 (cols 64-127), concurrent, accumulated over sc.
  - den rows gathered with one strided-partition DVE copy (+ -M_b add),
    reciprocal_approx_fast, cast to bf16, broadcast across partitions with
    a K=4 sel-matmul; normalize = one dual-PSUM DVE multiply per pair.
  - q-proj for tile t+1 is emitted before out-proj of tile t so the PE
    FIFO always has independent work while the softmax chain completes.
  - PSUM budget: ps_s [128,1024] x2 (4 banks) + pair x2 + qy x2 = 8 banks.
"""

import os

import numpy as np

try:
    import concourse.bass as bass
except ImportError:
    import sys

    sys.path.insert(0, "/opt/trn_rl_repo")
    import concourse.bass as bass

from contextlib import ExitStack

import concourse.mybir as mybir
import concourse.tile as tile
from concourse.bass import ts

B, N, S, D, H = 8, 4096, 512, 512, 8
HD = D // H  # 64
SCALE = HD**-0.5
P = 128
IC = D // P  # 4 chunks of the contraction/feature dims
SC = S // P  # 4 chunks of the context length
NT = 512  # queries per outer tile
NTILES = N // NT  # 8
NSUB = NT // P  # 4

f32 = mybir.dt.float32

MMDT_NAME = os.environ.get("KMMDT", "bfloat16")


def _np_mm(mmdt):
    return np.dtype(mybir.dt.np(mmdt))


def _split_multi_waits(nc: bass.Bass) -> None:
    """This walrus toolchain accepts at most ONE sync-wait per instruction
    ("Too many sync wait commands" in setupSyncWait, seen for MM/LW, NoOp,
    and DMA structs alike). Hoist all but the last wait of any instruction
    onto a chain of same-engine InstNoOps spliced immediately before it —
    same program position, so synchronization semantics are unchanged."""
    eng_map = {
        mybir.EngineType.PE: lambda: nc.tensor,
        mybir.EngineType.Activation: lambda: nc.scalar,
        mybir.EngineType.DVE: lambda: nc.vector,
        mybir.EngineType.Pool: lambda: nc.gpsimd,
        mybir.EngineType.SP: lambda: nc.sync,
    }
    for fn in nc.m.functions:
        blocks = fn.blocks
        for bb in blocks:
            insts = list(bb.instructions)
            out = []
            changed = False
            for inst in insts:
                si = inst.sync_info
                if (
                    si is not None
                    and len(si.on_wait) > 1
                    and inst.engine in eng_map
                ):
                    waits = list(si.on_wait)
                    for w in waits[:-1]:  # one nop per excess wait
                        nop = eng_map[inst.engine]().nop(nofuse=True).ins
                        # the nop was appended to whatever block is current;
                        # strip it from there before splicing it in place
                        for bb2 in blocks:
                            lst = list(bb2.instructions)
                            if any(x.name == nop.name for x in lst):
                                bb2.instructions = [
                                    x for x in lst if x.name != nop.name
                                ]
                                if bb2 is bb:
                                    insts = [
                                        x for x in insts if x.name != nop.name
                                    ]
                        nop.sync_info = mybir.SyncInfo(
                            on_wait=[w], on_update=[]
                        )
                        out.append(nop)
                    inst.sync_info = mybir.SyncInfo(
                        on_wait=waits[-1:], on_update=list(si.on_update)
                    )
                    changed = True
                out.append(inst)
            if changed:
                bb.instructions = out


def _build_nc(mmdt_name: str, has_bq, has_bk, has_bv, has_bp) -> bass.Bass:
    mmdt = getattr(mybir.dt, mmdt_name)
    nc = bass.Bass()

    xT = nc.dram_tensor("xT", [D, N], mmdt, kind="ExternalInput")
    ctxT = nc.dram_tensor("ctxT", [D, S], mmdt, kind="ExternalInput")
    wqT = nc.dram_tensor("wqT", [D, D], mmdt, kind="ExternalInput")
    wkT = nc.dram_tensor("wkT", [D, D], mmdt, kind="ExternalInput")
    wvT = nc.dram_tensor("wvT", [D, D], mmdt, kind="ExternalInput")
    wpT = nc.dram_tensor("wpT", [D, D], mmdt, kind="ExternalInput")
    bq = nc.dram_tensor("bq", [D, 1], f32, kind="ExternalInput")
    bk = nc.dram_tensor("bk", [D, 1], f32, kind="ExternalInput")
    bv = nc.dram_tensor("bv", [1, D], mmdt, kind="ExternalInput")
    bp = nc.dram_tensor("bp", [1, D], mmdt, kind="ExternalInput")
    mneg = nc.dram_tensor("mneg", [2, NT], mmdt, kind="ExternalInput")
    seld = nc.dram_tensor("seld", [2, P, P], mmdt, kind="ExternalInput")
    y = nc.dram_tensor("y", [N, D], mmdt, kind="ExternalOutput")

    ch = lambda dram: dram.rearrange("(c p) o -> p c o", p=P)  # [P, IC, D]

    with tile.TileContext(nc) as tc, ExitStack() as ctx:
        const = ctx.enter_context(tc.tile_pool(name="const", bufs=1))
        work = ctx.enter_context(tc.tile_pool(name="work", bufs=3))
        epool = ctx.enter_context(tc.tile_pool(name="epool", bufs=20))
        dpool = ctx.enter_context(tc.tile_pool(name="dpool", bufs=4))
        ypool = ctx.enter_context(tc.tile_pool(name="ypool", bufs=4))
        psum = ctx.enter_context(tc.tile_pool(name="psum", bufs=1, space="PSUM"))

        # ---- persistent tiles -------------------------------------------
        wq_t = const.tile([P, IC, D], mmdt)
        x0_t = const.tile([P, IC, NT], mmdt)  # tile-0 x, loaded first
        wk_t = const.tile([P, IC, D], mmdt)
        wv_t = const.tile([P, IC, D], mmdt)
        wp_t = const.tile([P, IC, D], mmdt)
        ctx_t = const.tile([P, IC, S], mmdt)
        selA_t = const.tile([P, P], mmdt)
        selB_t = const.tile([P, P], mmdt)
        mneg_t = const.tile([2, NT], mmdt)
        ones1_t = const.tile([P, 1], mmdt)
        ones2_t = const.tile([2, 1], mmdt)
        # wq + x0 first so q-proj can start while the rest stream in
        nc.sync.dma_start(wq_t[:], ch(wqT))
        nc.sync.dma_start(
            x0_t[:], xT[:, 0:NT].rearrange("(c p) n -> p c n", p=P)
        )
        nc.sync.dma_start(ctx_t[:], ch(ctxT))
        nc.sync.dma_start(wk_t[:], ch(wkT))
        nc.sync.dma_start(wv_t[:], ch(wvT))
        nc.sync.dma_start(wp_t[:], ch(wpT))
        nc.sync.dma_start(selA_t[:], seld[0])
        nc.sync.dma_start(selB_t[:], seld[1])
        nc.sync.dma_start(mneg_t[:], mneg[:])
        nc.vector.memset(ones1_t[:], 1.0)
        nc.vector.memset(ones2_t[:], 1.0)

        if has_bq:
            bq_t = const.tile([P, IC, 1], f32)
            nc.sync.dma_start(bq_t[:], bq.rearrange("(c p) o -> p c o", p=P))
        if has_bk:
            bk_t = const.tile([P, IC, 1], f32)
            nc.sync.dma_start(bk_t[:], bk.rearrange("(c p) o -> p c o", p=P))
        if has_bv:
            bv_t = const.tile([P, D], mmdt)
            nc.sync.dma_start(bv_t[:], bv.to_broadcast((P, D)))
        if has_bp:
            bp_t = const.tile([P, D], mmdt)
            nc.sync.dma_start(bp_t[:], bp.to_broadcast((P, D)))

        kT_t = const.tile([P, IC, S], mmdt)  # feature-major keys
        v_t = const.tile([P, SC, H, HD], mmdt)  # token-major values

        # ---- q-proj helper ----------------------------------------------
        def emit_qproj(x_tile, q_tile):
            for oc in range(IC):
                ps = psum.tile([P, NT], f32, tag="ps_qy", bufs=2)
                for i in range(IC):
                    nc.tensor.matmul(
                        ps[:],
                        wq_t[:, i, ts(oc, P)],
                        x_tile[:, i, :],
                        start=(i == 0),
                        stop=(i == IC - 1),
                    )
                if has_bq:
                    nc.vector.tensor_scalar_add(
                        q_tile[:, oc, :], ps[:], bq_t[:, oc, :]
                    )
                else:
                    nc.vector.tensor_copy(q_tile[:, oc, :], ps[:])

        # q-proj for tile 0 first: PE work that only needs wq + x0
        qT_cur = work.tile([P, IC, NT], mmdt, tag="qT")
        emit_qproj(x0_t, qT_cur)

        # ---- kv projections (once per core) -----------------------------
        for kc in range(IC):  # dk chunks -> kT
            ps = psum.tile([P, S], f32, tag="ps_qy", bufs=2)
            for i in range(IC):
                nc.tensor.matmul(
                    ps[:],
                    wk_t[:, i, ts(kc, P)],
                    ctx_t[:, i, :],
                    start=(i == 0),
                    stop=(i == IC - 1),
                )
            if has_bk:
                nc.vector.tensor_scalar_add(kT_t[:, kc, :], ps[:], bk_t[:, kc, :])
            else:
                nc.vector.tensor_copy(kT_t[:, kc, :], ps[:])

        for sc in range(SC):  # s chunks -> v (token-major)
            ps = psum.tile([P, D], f32, tag="ps_qy", bufs=2)
            for i in range(IC):
                nc.tensor.matmul(
                    ps[:],
                    ctx_t[:, i, ts(sc, P)],
                    wv_t[:, i, :],
                    start=(i == 0),
                    stop=(i == IC - 1),
                )
            if has_bv:
                nc.vector.tensor_add(v_t[:, sc, :, :], ps[:], bv_t[:])
            else:
                nc.vector.tensor_copy(v_t[:, sc, :, :], ps[:])

        # ---- main loop over query tiles ---------------------------------
        # Software-pipelined one tile deep: out-proj of tile t-1 is emitted
        # after the scores of tile t, so the PE FIFO never stalls on the
        # den/recip/broadcast/normalize tail of the current tile.
        prev = None  # (ot_t of previous tile, its tile index)
        for t in range(NTILES):
            es = {}
            rdens = {}
            ot_t = work.tile([P, IC, NT], mmdt, tag="ot")
            if t + 1 < NTILES:
                x_next = work.tile([P, IC, NT], mmdt, tag="xT")
                nc.sync.dma_start(
                    x_next[:],
                    xT[:, ts(t + 1, NT)].rearrange("(c p) n -> p c n", p=P),
                )

            # scores + exp for the whole tile (16 two-head groups)
            for c in range(IC):
                for sc in range(SC):
                    ps_s = psum.tile([P, 2 * NT], f32, tag="ps_s", bufs=2)
                    for par in (0, 1):  # PE row groups 0-63 / 64-127
                        pslc = slice(par * HD, (par + 1) * HD)
                        nc.tensor.matmul(
                            ps_s[:, par * NT : (par + 1) * NT],
                            kT_t[pslc, c, ts(sc, P)],
                            qT_cur[pslc, c, :],
                            start=True,
                            stop=True,
                        )
                    e = epool.tile([P, 2 * NT], mmdt, tag="e")
                    nc.scalar.activation(
                        e[:],
                        ps_s[:],
                        mybir.ActivationFunctionType.Exp,
                        scale=SCALE,
                    )
                    es[c, sc] = e

            # out-proj of the PREVIOUS tile: ready PE work while exps drain
            if prev is not None:
                pot, pt = prev
                for ns in range(NSUB):
                    ps_y = psum.tile([P, D], f32, tag="ps_qy", bufs=2)
                    for c in range(IC):
                        nc.tensor.matmul(
                            ps_y[:],
                            pot[:, c, ts(ns, P)],
                            wp_t[:, c, :],
                            start=(c == 0),
                            stop=(c == IC - 1),
                        )
                    y_t = ypool.tile([P, D], mmdt, tag="y")
                    if has_bp:
                        nc.vector.tensor_add(y_t[:], ps_y[:], bp_t[:])
                    else:
                        nc.vector.tensor_copy(y_t[:], ps_y[:])
                    nc.sync.dma_start(
                        y[pt * NT + ns * P : pt * NT + (ns + 1) * P, :], y_t[:]
                    )

            # q-proj for tile t+1: more ready PE work
            if t + 1 < NTILES:
                qT_next = work.tile([P, IC, NT], mmdt, tag="qT")
                emit_qproj(x_next, qT_next)

            # denominators + PV + normalize per head-quad
            for quad in range(2):
                ps_den = psum.tile([P, NT], f32, tag="ps_qy", bufs=2)
                nc.vector.memset(ps_den[:], 0.0)
                for sc in range(SC):
                    for j in range(4):
                        h = 4 * quad + j
                        nc.tensor.matmul(
                            ps_den[32 * j : 32 * j + 1, :],
                            ones1_t[:],
                            es[h // 2, sc][:, (h % 2) * NT : (h % 2 + 1) * NT],
                            start=(sc == 0),
                            stop=False,
                            tile_position=(0, 32 * j),
                        )
                for j in range(4):
                    # subtract masked count: -M_b = ones2.T @ (-M_b/2 rows)
                    nc.tensor.matmul(
                        ps_den[32 * j : 32 * j + 1, :],
                        ones2_t[:],
                        mneg_t[:],
                        start=False,
                        stop=True,
                        tile_position=(0, 32 * j),
                    )
                den_sb = dpool.tile([P, NT], f32, tag="den")
                nc.vector.tensor_scalar_max(den_sb[:], ps_den[:], 1e-20)
                rdenf = dpool.tile([P, NT], f32, tag="rdenf")
                nc.vector.reciprocal(rdenf[:], den_sb[:])
                rden = dpool.tile([P, NT], mmdt, tag="rden")
                nc.vector.tensor_copy(rden[:], rdenf[:])
                rdens[quad] = rden

                for c in (2 * quad, 2 * quad + 1):
                    ps_pair = psum.tile([P, NT], f32, tag="ps_pair", bufs=2)
                    for sc in range(SC):
                        for par in (0, 1):
                            nc.tensor.matmul(
                                ps_pair[par * HD : (par + 1) * HD, :],
                                v_t[:, sc, 2 * c + par, :],
                                es[c, sc][:, par * NT : (par + 1) * NT],
                                start=(sc == 0),
                                stop=(sc == SC - 1),
                            )
                    ps_bc = psum.tile([P, NT], f32, tag="ps_qy", bufs=2)
                    nc.tensor.matmul(
                        ps_bc[:],
                        (selA_t if c % 2 == 0 else selB_t)[:],
                        rdens[quad][:],
                        start=True,
                        stop=True,
                    )
                    bc_sb = dpool.tile([P, NT], mmdt, tag="bc")
                    nc.vector.tensor_copy(bc_sb[:], ps_bc[:])
                    nc.vector.tensor_mul(ot_t[:, c, :], ps_pair[:], bc_sb[:])

            prev = (ot_t, t)
            if t + 1 < NTILES:
                qT_cur = qT_next

        # drain: out-proj of the last tile
        pot, pt = prev
        for ns in range(NSUB):
            ps_y = psum.tile([P, D], f32, tag="ps_qy", bufs=2)
            for c in range(IC):
                nc.tensor.matmul(
                    ps_y[:],
                    pot[:, c, ts(ns, P)],
                    wp_t[:, c, :],
                    start=(c == 0),
                    stop=(c == IC - 1),
                )
            y_t = ypool.tile([P, D], mmdt, tag="y")
            if has_bp:
                nc.vector.tensor_add(y_t[:], ps_y[:], bp_t[:])
            else:
                nc.vector.tensor_copy(y_t[:], ps_y[:])
            nc.sync.dma_start(
                y[pt * NT + ns * P : pt * NT + (ns + 1) * P, :], y_t[:]
            )

    _split_multi_waits(nc)
    return nc


_NC_CACHE: dict = {}


def _get_nc(flags):
    if flags not in _NC_CACHE:
        _NC_CACHE[flags] = _build_nc(*flags)
    return _NC_CACHE[flags]


def _prep_in_maps(x, context, context_mask, wq, bq, wkv, bkv, wp, bp, mmdt_name=None):
    if mmdt_name is None:
        mmdt_name = MMDT_NAME
    np_mm = _np_mm(getattr(mybir.dt, mmdt_name))
    cvt = lambda a: np.ascontiguousarray(a).astype(np_mm, copy=False)
    wqT = cvt(wq.T)
    wkT = cvt(wkv[:D].T)
    wvT = cvt(wkv[D:].T)
    wpT = cvt(wp.T)
    bq_c = np.ascontiguousarray(bq.reshape(D, 1), dtype=np.float32)
    bk_c = np.ascontiguousarray(bkv[:D].reshape(D, 1), dtype=np.float32)
    bv_r = cvt(bkv[D:].reshape(1, D))
    bp_r = cvt(bp.reshape(1, D))
    sel = np.zeros((2, P, P), np.float32)
    sel[0, 0, 0:HD] = 1.0   # even pair of a quad: den rows 0 / 32
    sel[0, 32, HD:P] = 1.0
    sel[1, 64, 0:HD] = 1.0  # odd pair of a quad: den rows 64 / 96
    sel[1, 96, HD:P] = 1.0
    sel = cvt(sel)
    flags = (
        mmdt_name,
        bool(np.any(bq != 0)),
        bool(np.any(bkv[:D] != 0)),
        bool(np.any(bkv[D:] != 0)),
        bool(np.any(bp != 0)),
    )
    in_maps = []
    for b in range(B):
        ctx0 = np.where(context_mask[b][:, None], 0.0, context[b]).astype(
            context.dtype
        )
        m_b = float(context_mask[b].sum())
        in_maps.append(
            {
                "xT": cvt(x[b].T),
                "ctxT": cvt(ctx0.T),
                "wqT": wqT,
                "wkT": wkT,
                "wvT": wvT,
                "wpT": wpT,
                "bq": bq_c,
                "bk": bk_c,
                "bv": bv_r,
                "bp": bp_r,
                "mneg": cvt(np.full((2, NT), -m_b / 2.0, np.float32)),
                "seld": sel,
            }
        )
    return in_maps, flags


def kernel(x, context, context_mask, wq, bq, wkv, bkv, wp, bp):
    from concourse.bass_utils import run_bass_kernel_spmd

    in_maps, flags = _prep_in_maps(
        x, context, context_mask, wq, bq, wkv, bkv, wp, bp
    )
    nc = _get_nc(flags)
    res = run_bass_kernel_spmd(nc, in_maps, list(range(B)))
    return np.stack(
        [np.asarray(res.results[b]["y"]).astype(np.float32) for b in range(B)],
        axis=0,
    )
